# revision 10
# baseline (speedup 1.0000x reference)
"""MinLSTM layer on 8 Trainium2 NeuronCores.

Math (equivalent to the log-space reference, done in linear space):
    f_pre = x @ W_f.T + b_f ; i_pre = x @ W_i.T + b_i ; h_pre = x @ W_h.T + b_h
    sf = sigmoid(f_pre) ; si = sigmoid(i_pre)
    f = sf / (sf + si)                       # normalized forget gate
    i = 1 - f                                # = si / (sf + si)
    g = max(sigmoid(h_pre), h_pre + 0.5)     # == exp(log_g), exactly
    h_t = f_t * h_{t-1} + i_t * g_t,  h_0 = 1
The gates satisfy f in (0,1), g > 0, so h stays in a tame range and the
recurrence is numerically stable in fp32 (max rel err vs the fp32 log-space
reference ~6e-4 = the reference's own fp32 noise floor).

Sharding: 8 cores = batch(4) x hidden-halves(2). Core c handles batch b=c//2,
hidden slice [(c%2)*512, (c%2+1)*512). No cross-core communication; the scan
runs along T inside each core via the DVE TensorTensorScan instruction
(state = f*state - mv per step, mv = (f-1)*g = -i*g).

Device layout: gates computed as [h_part, t_free] via out = W_sliceT.T @ xT;
host pre-transposes x and W (numpy) and re-transposes the [512, 4096] per-core
output back to [T, Dh]. Matmuls run in 512-wide t-chunks (one PSUM bank);
elementwise+scan run in up-to-1024-wide super-chunks to amortize DVE fixed
overhead, with the scan carry passed as the previous chunk's last column.

Scheduling notes:
- x and W live in per-k tiles (contraction slices) so the PE's dependency on
  each matmul is one 256KB DMA, not a whole 2MB tensor: at startup the PE
  chases the HBM stream (~390 GB/s) instead of idling for all weights.
- First super-chunk is gate-major (f for all h-tiles, then i, then h) in DMA
  priority order x0 -> W_f -> x1 -> W_i -> W_h; later chunks are h-tile-major.
- Warmup matmuls on a zeroed scratch tile run during the initial DMA wait so
  the PE's HAM clock gate is already at 2.4 GHz when real matmuls start.
"""

import sys

for _p in ("/opt/trn_rl_repo",):
    if _p not in sys.path:
        sys.path.append(_p)

import numpy as np

import concourse.bass as bass
import concourse.tile as tile
from concourse import bacc, mybir
from concourse.bass_utils import run_bass_kernel_spmd

B, T, DIN, DH = 4, 4096, 1024, 1024
N_CORES = 8
HSH = DH // 2          # 512 hidden channels per core
P = 128                # partitions
KT = DIN // P          # 8 contraction tiles
NT = 512               # matmul t-chunk (free dim, one PSUM bank)
IT = HSH // P          # 4 h-tiles per core
# elementwise/scan super-chunks (start, length); tail chunks smaller to
# shrink the end-of-kernel drain
CHUNKS = [(0, 1024), (1024, 1024), (2048, 1024), (3072, 512), (3584, 512)]

# float32r streams fp32 operands through the PE at bf16 rate when the moving
# free dim >= 256. Measured (K=128): mean rel err ~1e-3 vs fp64, ~16x better
# than bf16. Fallbacks: mybir.dt.float32 (4x slower, exact) / bfloat16.
MM_DT = mybir.dt.float32r

_COMPILED = None


def _build():
    AF = mybir.ActivationFunctionType
    OP = mybir.AluOpType
    f32 = mybir.dt.float32

    nc = bacc.Bacc("TRN2", target_bir_lowering=False, debug=False)

    xT = nc.dram_tensor("xT", [DIN, T], MM_DT, kind="ExternalInput").ap()
    wd = {g: nc.dram_tensor(f"w{g}", [DIN, HSH], MM_DT, kind="ExternalInput").ap()
          for g in ("f", "i", "h")}
    # packed per-partition scalars: [b_f | b_i | b_h | b_h+0.5], each (128, IT)
    biases = nc.dram_tensor("biases", [P, 4 * IT], f32, kind="ExternalInput").ap()
    out = nc.dram_tensor("out", [HSH, T], f32, kind="ExternalOutput").ap()

    # DRAM views: (KT*P, n) -> [p, k, n]
    xT_v = xT.rearrange("(k p) t -> p k t", p=P)
    w_v = {g: w.rearrange("(k p) h -> p k h", p=P) for g, w in wd.items()}

    with tile.TileContext(nc) as tc:
        with (
            tc.tile_pool(name="wpool", bufs=1) as wpool,
            tc.tile_pool(name="bpool", bufs=1) as bpool,
            tc.tile_pool(name="xpool", bufs=3) as xpool,
            tc.tile_pool(name="psum", bufs=7, space="PSUM") as pspool,
            tc.tile_pool(name="work", bufs=4) as work,
            tc.tile_pool(name="hpool", bufs=2) as hpool,
        ):
            bias_t = bpool.tile([P, 4 * IT], f32, tag="bias")
            nc.sync.dma_start(out=bias_t[:], in_=biases[:])

            # per-k weight tiles, resident all kernel
            wt = {g: [wpool.tile([P, HSH], MM_DT, tag=f"w{g}{k}", name=f"w{g}{k}_t")
                      for k in range(KT)] for g in ("f", "i", "h")}

            def dma_w(g):
                for k in range(KT):
                    nc.sync.dma_start(out=wt[g][k][:], in_=w_v[g][:, k, :])

            def x_tile(t0):
                """One [P, KT, NT] tile: all contraction slices of a t-chunk."""
                xc = xpool.tile([P, KT, NT], MM_DT, tag="xc", name="xc_t")
                nc.sync.dma_start(out=xc[:], in_=xT_v[:, :, t0:t0 + NT])
                return xc

            def bias_ap(kind, i):
                return bias_t[:, kind * IT + i:kind * IT + i + 1]

            def chain(i, sf, si, sg, gt, J, t0, ne):
                """Normalize gates, build -i*g, scan, and store chunk."""
                nc.vector.tensor_add(si[:], sf[:], si[:])          # s = sf+si
                r = work.tile([P, ne], f32, tag="sg", name="r_t")  # sg slot free
                nc.vector.reciprocal_approx_fast(out=r[:], in_=si[:])
                nc.vector.tensor_mul(sf[:], sf[:], r[:])           # f
                nc.vector.scalar_tensor_tensor(                    # mv = (f-1)*g
                    gt[:], sf[:], 1.0, gt[:], op0=OP.subtract, op1=OP.mult)
                hc = hpool.tile([P, ne], f32, tag=f"h{i}", name=f"h{i}_t")
                init = 1.0 if J == 0 else hprev[i][:, -1:]
                nc.vector.tensor_tensor_scan(
                    hc[:], sf[:], gt[:], init, op0=OP.mult, op1=OP.subtract)
                hprev[i] = hc
                nc.sync.dma_start(out=out[i * P:(i + 1) * P, t0:t0 + ne], in_=hc[:])

            hprev = [None] * IT
            hsls = [slice(i * P, (i + 1) * P) for i in range(IT)]

            # Warm up the PE clock gate while input DMAs stream: ~30 matmuls
            # on a zeroed scratch tile (results discarded). No DMA deps, so
            # they start right after the engine preamble.
            scratch = bpool.tile([P, NT], MM_DT, tag="scratch")
            nc.vector.memset(scratch[:].bitcast(mybir.dt.uint32), 0)
            pswarm = pspool.tile([P, NT], f32, tag="warm", name="pswarm_t", bufs=1)
            for _ in range(30):
                nc.tensor.matmul(pswarm[:], lhsT=scratch[:, :P],
                                 rhs=scratch[:], start=True, stop=True)

            # ---- J0: gate-major, k-outer; PE chases the input DMA stream ----
            t0, ne = CHUNKS[0]
            nhalf = ne // NT
            # DMA priority order: x(h0), W_f (k-sliced), x(h1), W_i, W_h
            xc0 = x_tile(t0)
            dma_w("f")
            xcs = [xc0] + [x_tile(t0 + h * NT) for h in range(1, nhalf)]
            dma_w("i")
            dma_w("h")

            sf = [work.tile([P, ne], f32, tag="sf", name="sf_t") for _ in range(IT)]
            si = [work.tile([P, ne], f32, tag="si", name="si_t") for _ in range(IT)]
            sg = [work.tile([P, ne], f32, tag="sg", name="sg_t") for _ in range(IT)]
            gt = [work.tile([P, ne], f32, tag="gt", name="gt_t") for _ in range(IT)]
            for gate, dsts, bk in (("f", sf, 0), ("i", si, 1), ("h", sg, 2)):
                for half in range(nhalf):
                    esl = slice(half * NT, (half + 1) * NT)
                    psts = [pspool.tile([P, NT], f32, tag="ps", name="ps_t")
                            for _ in range(IT)]
                    for k in range(KT):
                        for pst, hsl in zip(psts, hsls):
                            nc.tensor.matmul(
                                pst[:], lhsT=wt[gate][k][:, hsl],
                                rhs=xcs[half][:, k, :],
                                start=(k == 0), stop=(k == KT - 1))
                    for i in range(IT):
                        nc.scalar.activation(dsts[i][:, esl], psts[i][:], AF.Sigmoid,
                                             bias=bias_ap(bk, i), scale=1.0)
                        if gate == "h":
                            nc.vector.scalar_tensor_tensor(
                                gt[i][:, esl], psts[i][:], bias_ap(3, i),
                                sg[i][:, esl], op0=OP.add, op1=OP.max)
            for i in range(IT):
                chain(i, sf[i], si[i], sg[i], gt[i], 0, t0, ne)

            # ---- J1+: h-tile-major units ----
            for J, (t0, ne) in enumerate(CHUNKS[1:], start=1):
                nhalf = ne // NT
                xcs = [x_tile(t0 + h * NT) for h in range(nhalf)]
                for i in range(IT):
                    hsl = hsls[i]
                    sf = work.tile([P, ne], f32, tag="sf", name="sf_t")
                    si = work.tile([P, ne], f32, tag="si", name="si_t")
                    sg = work.tile([P, ne], f32, tag="sg", name="sg_t")
                    gt = work.tile([P, ne], f32, tag="gt", name="gt_t")
                    for half in range(nhalf):
                        esl = slice(half * NT, (half + 1) * NT)
                        for gate, dst, bk in (("f", sf, 0), ("i", si, 1),
                                              ("h", sg, 2)):
                            pst = pspool.tile([P, NT], f32, tag="ps", name="ps_t")
                            for k in range(KT):
                                nc.tensor.matmul(
                                    pst[:], lhsT=wt[gate][k][:, hsl],
                                    rhs=xcs[half][:, k, :],
                                    start=(k == 0), stop=(k == KT - 1))
                            nc.scalar.activation(dst[:, esl], pst[:],
                                                 AF.Sigmoid, bias=bias_ap(bk, i),
                                                 scale=1.0)
                            if gate == "h":
                                nc.vector.scalar_tensor_tensor(
                                    gt[:, esl], pst[:], bias_ap(3, i),
                                    sg[:, esl], op0=OP.add, op1=OP.max)
                    chain(i, sf, si, sg, gt, J, t0, ne)

    nc.compile()
    return nc


def _in_maps(x, W_f, b_f, W_i, b_i, W_h, b_h):
    x = np.asarray(x, np.float32)
    wT = {g: np.ascontiguousarray(np.asarray(w, np.float32).T)
          for g, w in (("f", W_f), ("i", W_i), ("h", W_h))}
    bs = {g: np.asarray(b, np.float32) for g, b in (("f", b_f), ("i", b_i), ("h", b_h))}

    maps = []
    for c in range(N_CORES):
        b, hh = divmod(c, 2)
        hsl = slice(hh * HSH, (hh + 1) * HSH)
        bias_pack = np.concatenate([
            bs["f"][hsl].reshape(IT, P).T,
            bs["i"][hsl].reshape(IT, P).T,
            bs["h"][hsl].reshape(IT, P).T,
            (bs["h"][hsl] + 0.5).reshape(IT, P).T,
        ], axis=1)
        maps.append({
            "xT": np.ascontiguousarray(x[b].T),
            "wf": np.ascontiguousarray(wT["f"][:, hsl]),
            "wi": np.ascontiguousarray(wT["i"][:, hsl]),
            "wh": np.ascontiguousarray(wT["h"][:, hsl]),
            "biases": np.ascontiguousarray(bias_pack, dtype=np.float32),
        })
    return maps


def kernel(x, W_f, b_f, W_i, b_i, W_h, b_h):
    global _COMPILED
    if _COMPILED is None:
        _COMPILED = _build()
    nc = _COMPILED

    res = run_bass_kernel_spmd(
        nc, _in_maps(x, W_f, b_f, W_i, b_i, W_h, b_h), list(range(N_CORES)))

    full = np.empty((B, T, DH), np.float32)
    for c in range(N_CORES):
        b, hh = divmod(c, 2)
        full[b, :, hh * HSH:(hh + 1) * HSH] = res.results[c]["out"].T
    return full


# revision 13
# speedup vs baseline: 1.0390x; 1.0390x over previous
"""MinLSTM layer on 8 Trainium2 NeuronCores.

Math (equivalent to the log-space reference, done in linear space):
    f_pre = x @ W_f.T + b_f ; i_pre = x @ W_i.T + b_i ; h_pre = x @ W_h.T + b_h
    sf = sigmoid(f_pre) ; si = sigmoid(i_pre)
    f = sf / (sf + si)                       # normalized forget gate
    i = 1 - f                                # = si / (sf + si)
    g = max(sigmoid(h_pre), h_pre + 0.5)     # == exp(log_g), exactly
    h_t = f_t * h_{t-1} + i_t * g_t,  h_0 = 1
The gates satisfy f in (0,1), g > 0, so h stays in a tame range and the
recurrence is numerically stable in fp32 (max rel err vs the fp32 log-space
reference ~6e-4 = the reference's own fp32 noise floor).

Sharding: 8 cores = batch(4) x hidden-halves(2). Core c handles batch b=c//2,
hidden slice [(c%2)*512, (c%2+1)*512). No cross-core communication; the scan
runs along T inside each core via the DVE TensorTensorScan instruction
(state = f*state - mv per step, mv = (f-1)*g = -i*g).

Device layout: gates computed as [h_part, t_free] via out = W_sliceT.T @ xT;
host pre-transposes x and W (numpy) and re-transposes the [512, 4096] per-core
output back to [T, Dh]. Matmuls run in 512-wide t-chunks (one PSUM bank);
elementwise+scan run in up-to-1024-wide super-chunks to amortize DVE fixed
overhead, with the scan carry passed as the previous chunk's last column.

Scheduling notes:
- x and W live in per-k tiles (contraction slices) so the PE's dependency on
  each matmul is one 256KB DMA, not a whole 2MB tensor: at startup the PE
  chases the HBM stream (~390 GB/s) instead of idling for all weights.
- First super-chunk is gate-major (f for all h-tiles, then i, then h) in DMA
  priority order x0 -> W_f -> x1 -> W_i -> W_h; later chunks are h-tile-major.
- Later chunks are h-tile-major with one fused x DMA per 512-chunk.
"""

import sys

for _p in ("/opt/trn_rl_repo",):
    if _p not in sys.path:
        sys.path.append(_p)

import numpy as np

import concourse.bass as bass
import concourse.tile as tile
from concourse import bacc, mybir
from concourse.bass_utils import run_bass_kernel_spmd

B, T, DIN, DH = 4, 4096, 1024, 1024
N_CORES = 8
HSH = DH // 2          # 512 hidden channels per core
P = 128                # partitions
KT = DIN // P          # 8 contraction tiles
NT = 512               # matmul t-chunk (free dim, one PSUM bank)
IT = HSH // P          # 4 h-tiles per core
# elementwise/scan super-chunks (start, length); tail chunks smaller to
# shrink the end-of-kernel drain
CHUNKS = [(0, 1024), (1024, 1024), (2048, 1024), (3072, 512), (3584, 512)]

# float32r streams fp32 operands through the PE at bf16 rate when the moving
# free dim >= 256. Measured (K=128): mean rel err ~1e-3 vs fp64, ~16x better
# than bf16. Fallbacks: mybir.dt.float32 (4x slower, exact) / bfloat16.
MM_DT = mybir.dt.float32r

_COMPILED = None


def _build():
    AF = mybir.ActivationFunctionType
    OP = mybir.AluOpType
    f32 = mybir.dt.float32

    nc = bacc.Bacc("TRN2", target_bir_lowering=False, debug=False)

    xT = nc.dram_tensor("xT", [DIN, T], MM_DT, kind="ExternalInput").ap()
    wd = {g: nc.dram_tensor(f"w{g}", [DIN, HSH], MM_DT, kind="ExternalInput").ap()
          for g in ("f", "i", "h")}
    # packed per-partition scalars: [b_f | b_i | b_h | b_h+0.5], each (128, IT)
    biases = nc.dram_tensor("biases", [P, 4 * IT], f32, kind="ExternalInput").ap()
    out = nc.dram_tensor("out", [HSH, T], f32, kind="ExternalOutput").ap()

    # DRAM views: (KT*P, n) -> [p, k, n]
    xT_v = xT.rearrange("(k p) t -> p k t", p=P)
    w_v = {g: w.rearrange("(k p) h -> p k h", p=P) for g, w in wd.items()}

    with tile.TileContext(nc) as tc:
        with (
            tc.tile_pool(name="wpool", bufs=1) as wpool,
            tc.tile_pool(name="bpool", bufs=1) as bpool,
            tc.tile_pool(name="xpool", bufs=24) as xpool,
            tc.tile_pool(name="psum", bufs=7, space="PSUM") as pspool,
            tc.tile_pool(name="work", bufs=4) as work,
            tc.tile_pool(name="hpool", bufs=6) as hpool,
        ):
            bias_t = bpool.tile([P, 4 * IT], f32, tag="bias")
            nc.sync.dma_start(out=bias_t[:], in_=biases[:])

            # per-k weight tiles, resident all kernel
            wt = {g: [wpool.tile([P, HSH], MM_DT, tag=f"w{g}{k}", name=f"w{g}{k}_t")
                      for k in range(KT)] for g in ("f", "i", "h")}

            def dma_w(g):
                for k in range(KT):
                    nc.sync.dma_start(out=wt[g][k][:], in_=w_v[g][:, k, :])

            def x_ktiles(t0):
                """One [P, NT] tile per contraction slice k of a t-chunk."""
                xs = []
                for k in range(KT):
                    xk = xpool.tile([P, NT], MM_DT, tag="xk", name="xk_t")
                    nc.sync.dma_start(out=xk[:], in_=xT_v[:, k, t0:t0 + NT])
                    xs.append(xk)
                return xs

            def bias_ap(kind, i):
                return bias_t[:, kind * IT + i:kind * IT + i + 1]

            def chain(i, sf, si, sg, gt, J, t0, ne):
                """Normalize gates, build -i*g, scan, and store chunk."""
                nc.vector.tensor_add(si[:], sf[:], si[:])          # s = sf+si
                r = work.tile([P, ne], f32, tag="sg", name="r_t")  # sg slot free
                nc.vector.reciprocal_approx_fast(out=r[:], in_=si[:])
                nc.vector.tensor_mul(sf[:], sf[:], r[:])           # f
                nc.vector.scalar_tensor_tensor(                    # mv = (f-1)*g
                    gt[:], sf[:], 1.0, gt[:], op0=OP.subtract, op1=OP.mult)
                hc = hpool.tile([P, ne], f32, tag="h", name=f"h{i}_t")
                init = 1.0 if J == 0 else hprev[i][:, -1:]
                nc.vector.tensor_tensor_scan(
                    hc[:], sf[:], gt[:], init, op0=OP.mult, op1=OP.subtract)
                hprev[i] = hc
                nc.sync.dma_start(out=out[i * P:(i + 1) * P, t0:t0 + ne], in_=hc[:])

            hprev = [None] * IT
            hsls = [slice(i * P, (i + 1) * P) for i in range(IT)]

            # ---- J0: gate-major, k-outer; PE chases the input DMA stream ----
            t0, ne = CHUNKS[0]
            nhalf = ne // NT
            # priority order: (x_h0[k], W_f[k]) interleaved, x_h1, W_i, W_h
            xcs = [[xpool.tile([P, NT], MM_DT, tag="xk", name="xk_t")
                    for _ in range(KT)] for _ in range(nhalf)]
            for k in range(KT):
                nc.sync.dma_start(out=xcs[0][k][:], in_=xT_v[:, k, t0:t0 + NT])
                nc.sync.dma_start(out=wt["f"][k][:], in_=w_v["f"][:, k, :])
            for h in range(1, nhalf):
                th = t0 + h * NT
                for k in range(KT):
                    nc.sync.dma_start(out=xcs[h][k][:], in_=xT_v[:, k, th:th + NT])
            dma_w("i")
            dma_w("h")

            sf = [work.tile([P, ne], f32, tag="sf", name="sf_t") for _ in range(IT)]
            si = [work.tile([P, ne], f32, tag="si", name="si_t") for _ in range(IT)]
            sg = [work.tile([P, ne], f32, tag="sg", name="sg_t") for _ in range(IT)]
            gt = [work.tile([P, ne], f32, tag="gt", name="gt_t") for _ in range(IT)]
            for gate, dsts, bk in (("f", sf, 0), ("i", si, 1), ("h", sg, 2)):
                for half in range(nhalf):
                    esl = slice(half * NT, (half + 1) * NT)
                    psts = [pspool.tile([P, NT], f32, tag="ps", name="ps_t")
                            for _ in range(IT)]
                    for k in range(KT):
                        for pst, hsl in zip(psts, hsls):
                            nc.tensor.matmul(
                                pst[:], lhsT=wt[gate][k][:, hsl],
                                rhs=xcs[half][k][:],
                                start=(k == 0), stop=(k == KT - 1))
                    for i in range(IT):
                        nc.scalar.activation(dsts[i][:, esl], psts[i][:], AF.Sigmoid,
                                             bias=bias_ap(bk, i), scale=1.0)
                        if gate == "h":
                            nc.vector.scalar_tensor_tensor(
                                gt[i][:, esl], psts[i][:], bias_ap(3, i),
                                sg[i][:, esl], op0=OP.add, op1=OP.max)
            for i in range(IT):
                chain(i, sf[i], si[i], sg[i], gt[i], 0, t0, ne)

            # ---- J1+: h-tile-major units ----
            for J, (t0, ne) in enumerate(CHUNKS[1:], start=1):
                nhalf = ne // NT
                xcs = [x_ktiles(t0 + h * NT) for h in range(nhalf)]
                for i in range(IT):
                    hsl = hsls[i]
                    sf = work.tile([P, ne], f32, tag="sf", name="sf_t")
                    si = work.tile([P, ne], f32, tag="si", name="si_t")
                    sg = work.tile([P, ne], f32, tag="sg", name="sg_t")
                    gt = work.tile([P, ne], f32, tag="gt", name="gt_t")
                    for half in range(nhalf):
                        esl = slice(half * NT, (half + 1) * NT)
                        for gate, dst, bk in (("f", sf, 0), ("i", si, 1),
                                              ("h", sg, 2)):
                            pst = pspool.tile([P, NT], f32, tag="ps", name="ps_t")
                            for k in range(KT):
                                nc.tensor.matmul(
                                    pst[:], lhsT=wt[gate][k][:, hsl],
                                    rhs=xcs[half][k][:],
                                    start=(k == 0), stop=(k == KT - 1))
                            nc.scalar.activation(dst[:, esl], pst[:],
                                                 AF.Sigmoid, bias=bias_ap(bk, i),
                                                 scale=1.0)
                            if gate == "h":
                                nc.vector.scalar_tensor_tensor(
                                    gt[:, esl], pst[:], bias_ap(3, i),
                                    sg[:, esl], op0=OP.add, op1=OP.max)
                    chain(i, sf, si, sg, gt, J, t0, ne)

    nc.compile()
    return nc


def _in_maps(x, W_f, b_f, W_i, b_i, W_h, b_h):
    x = np.asarray(x, np.float32)
    wT = {g: np.ascontiguousarray(np.asarray(w, np.float32).T)
          for g, w in (("f", W_f), ("i", W_i), ("h", W_h))}
    bs = {g: np.asarray(b, np.float32) for g, b in (("f", b_f), ("i", b_i), ("h", b_h))}

    maps = []
    for c in range(N_CORES):
        b, hh = divmod(c, 2)
        hsl = slice(hh * HSH, (hh + 1) * HSH)
        bias_pack = np.concatenate([
            bs["f"][hsl].reshape(IT, P).T,
            bs["i"][hsl].reshape(IT, P).T,
            bs["h"][hsl].reshape(IT, P).T,
            (bs["h"][hsl] + 0.5).reshape(IT, P).T,
        ], axis=1)
        maps.append({
            "xT": np.ascontiguousarray(x[b].T),
            "wf": np.ascontiguousarray(wT["f"][:, hsl]),
            "wi": np.ascontiguousarray(wT["i"][:, hsl]),
            "wh": np.ascontiguousarray(wT["h"][:, hsl]),
            "biases": np.ascontiguousarray(bias_pack, dtype=np.float32),
        })
    return maps


def kernel(x, W_f, b_f, W_i, b_i, W_h, b_h):
    global _COMPILED
    if _COMPILED is None:
        _COMPILED = _build()
    nc = _COMPILED

    res = run_bass_kernel_spmd(
        nc, _in_maps(x, W_f, b_f, W_i, b_i, W_h, b_h), list(range(N_CORES)))

    full = np.empty((B, T, DH), np.float32)
    for c in range(N_CORES):
        b, hh = divmod(c, 2)
        full[b, :, hh * HSH:(hh + 1) * HSH] = res.results[c]["out"].T
    return full
